# revision 40
# baseline (speedup 1.0000x reference)
"""Trainium2 Bass kernel for nn_DFDgraph (gnn_message_passing).

Pipeline per batch element (one NeuronCore each, 8 total):
  x (2048, 288) --rfft-mag--> (2048, 145) --minmax+l2--> xn
  h = LN(relu(cat[xn @ Wd0, te_norm] @ We0))            (2048, 64)
  adj = relu((h * w) @ h^T)                             (2048, 2048)
  out = top10_row_mask(adj) / (rowsum_top10 + 1e-5)

Everything stays fp32: P(gap(v10,v11) < 1e-3) ~ 6.6% per row, so the
~1e-3 noise of f32r/bf16 matmuls flips enough top-k selections to blow
the error budget.

Phase 1 (row tiles of 128, groups of 4): PE transposes + DFT matmuls
against host-precomputed cos|sin matrices; one batched ACT square over
[128, 290]; group-batched minmax/l2/LN stats; elementwise work spread
across DVE/Pool/ACT; t_emb normalization fully hoisted out of the loop.
Stage C/D PSUM scratch is packed into one rotating 1-bank tile (pa/hT
and pb/pc share regions, ordered by within-tile deps).

Phase 2 per tile: 4 fp32 matmuls -> PSUM; the ONLY PSUM reader is a
plain ACT copy, so the two PSUM slots recycle fast and the PE never
stalls (pstate stays ramped). From the SBUF copy: per-quarter max8
gives 32 candidates (exact top-10 w.p. ~1-3e-4 miss/row), merge via
max8+match_replace+max8 -> exact top-16; den = sum(relu(top10)) + 1e-5
and thr = max(v10, tiny)/den are known BEFORE the select, so ACT
re-copies scaled by 1/den and the select (adjr >= thr)*adjr needs no
further scale pass. Select is split: DVE scalar_tensor_tensor on cols
0:CD, Pool mask+mult (stt is not supported on Pool; Pool cannot read
PSUM) on cols CD:N. relu is never materialized: negatives cannot pass
thr > 0, and rows with <10 positives are handled by den/thr clamping.

Known HW pitfalls baked in: tensor_tensor_reduce mis-executes on HW
(use ACT square+accum); fp32r matmul needs f32r-rounded producers and
is numerically unacceptable here anyway.
"""

import numpy as np
from contextlib import ExitStack

import concourse.bass as bass
import concourse.mybir as mybir
from concourse import bacc
from concourse import tile
from concourse import masks
from concourse.bass_utils import run_bass_kernel_spmd

F32 = mybir.dt.float32
AX = mybir.AxisListType
OP = mybir.AluOpType
AF = mybir.ActivationFunctionType

B, N, T, H, EMB, TOPK = 8, 2048, 288, 64, 24, 10
F = T // 2 + 1          # 145
P = 128                 # rows per tile
NT = N // P             # 16 tiles
G = 2                   # tiles per pipeline group
KC = 96                 # DFT contraction chunk (3 x 96 = 288)
NCORES = 8

_CACHE = {}


def _build():
    nc = bacc.Bacc("TRN2", target_bir_lowering=False, debug=False,
                   num_devices=NCORES)
    x_d = nc.declare_dram_parameter("x", [N, T], F32, isOutput=False)
    te_d = nc.declare_dram_parameter("t_emb", [N, EMB], F32, isOutput=False)
    cc_d = nc.declare_dram_parameter("ccos", [T, F], F32, isOutput=False)
    cs_d = nc.declare_dram_parameter("csin", [T, F], F32, isOutput=False)
    wd_d = nc.declare_dram_parameter("wd0", [F, H], F32, isOutput=False)
    we_d = nc.declare_dram_parameter("we0", [H + EMB, H], F32, isOutput=False)
    w_d = nc.declare_dram_parameter("w", [H, 1], F32, isOutput=False)
    out_d = nc.declare_dram_parameter("out", [N, N], F32, isOutput=True)

    with tile.TileContext(nc) as tc, ExitStack() as ctx:
        const = ctx.enter_context(tc.tile_pool(name="const", bufs=1))
        ident = const.tile([P, P], F32)
        masks.make_identity(nc, ident[:])
        ccs_sb = const.tile([KC, 3, 2 * F], F32)
        for c in range(3):
            nc.sync.dma_start(ccs_sb[:, c, 0:F], cc_d[c * KC:(c + 1) * KC, :])
            nc.sync.dma_start(ccs_sb[:, c, F:2 * F], cs_d[c * KC:(c + 1) * KC, :])
        wd_a = const.tile([P, H], F32)
        wd_b = const.tile([F - P, H], F32)
        we_sb = const.tile([H + EMB, H], F32)
        w_sb = const.tile([H, 1], F32)

        # persistent phase-1 results
        p1 = ctx.enter_context(tc.tile_pool(name="p1", bufs=1))
        hT_sb = p1.tile([H, N], F32)
        hTw_sb = p1.tile([H, N], F32)
        # [P, NT] stats, persistent
        st = ctx.enter_context(tc.tile_pool(name="stats", bufs=1))
        mx_s = st.tile([P, NT], F32)
        mn_s = st.tile([P, NT], F32)
        rd_s = st.tile([P, NT], F32)
        ssx_s = st.tile([P, NT], F32)
        rnx_s = st.tile([P, NT], F32)
        mxt_s = st.tile([P, NT], F32)
        mnt_s = st.tile([P, NT], F32)
        rdt_s = st.tile([P, NT], F32)
        sst_s = st.tile([P, NT], F32)
        rnt_s = st.tile([P, NT], F32)
        sums_s = st.tile([P, NT], F32)
        mean_s = st.tile([P, NT], F32)
        ssh_s = st.tile([P, NT], F32)
        rstd_s = st.tile([P, NT], F32)

        # t_emb pipeline hoisted out of the tile loop: one batched minmax +
        # l2 over [P, NT, EMB]; per-tile scalars applied on Pool.
        te_all = p1.tile([P, NT, EMB], F32)
        ten_all = p1.tile([P, NT, EMB], F32)
        for t in range(NT):
            nc.sync.dma_start(te_all[:, t, :], te_d[t * P:(t + 1) * P, :])
        nc.vector.tensor_reduce(mxt_s[:], te_all[:], axis=AX.X, op=OP.max)
        nc.vector.tensor_reduce(mnt_s[:], te_all[:], axis=AX.X, op=OP.min)
        nc.vector.scalar_tensor_tensor(rdt_s[:], mxt_s[:], 1.0, mnt_s[:],
                                       op0=OP.add, op1=OP.subtract)
        nc.vector.reciprocal(rdt_s[:], rdt_s[:])
        for t in range(NT):
            nc.gpsimd.tensor_scalar(ten_all[:, t, :], te_all[:, t, :],
                                    scalar1=mnt_s[:, t:t + 1],
                                    scalar2=rdt_s[:, t:t + 1],
                                    op0=OP.subtract, op1=OP.mult)
        sqt = p1.tile([P, NT, EMB], F32)
        nc.vector.tensor_tensor(sqt[:], ten_all[:], ten_all[:], op=OP.mult)
        nc.vector.tensor_reduce(sst_s[:], sqt[:], axis=AX.X, op=OP.add)
        nc.scalar.sqrt(sst_s[:], sst_s[:])
        nc.vector.reciprocal(rnt_s[:], sst_s[:])

        # group-cycled working buffers (bufs=2 -> group g+1 overlaps group g)
        p1ps = ExitStack()
        gp = p1ps.enter_context(tc.tile_pool(name="gp", bufs=4))
        ps_a = p1ps.enter_context(tc.tile_pool(name="ps_a", bufs=2, space="PSUM"))
        ps_b = p1ps.enter_context(tc.tile_pool(name="ps_b", bufs=2, space="PSUM"))

        for g in range(NT // G):
            t0 = g * G
            sl = slice(t0, t0 + G)
            # ---- stage A: load x, transpose, DFT, |.|^2, mag ----
            mag = gp.tile([P, G, F], F32, tag="mag")
            for j in range(G):
                t = t0 + j
                x_t = gp.tile([P, T], F32, tag="x", bufs=3)
                nc.sync.dma_start(x_t[:], x_d[t * P:(t + 1) * P, :])
                xTp = ps_a.tile([KC, 3, P], F32, tag="xT_ps")
                for c in range(3):
                    nc.tensor.transpose(xTp[:, c, :], x_t[:, c * KC:(c + 1) * KC],
                                        ident[:])
                xT = gp.tile([KC, 3, P], F32, tag="xT", bufs=3)
                nc.scalar.activation(xT[:], xTp[:], AF.Copy)
                ri_ps = ps_a.tile([P, 2 * F], F32, tag="ri_ps", bufs=3)
                for c in range(3):
                    nc.tensor.matmul(ri_ps[:], lhsT=xT[:, c, :], rhs=ccs_sb[:, c, :],
                                     start=(c == 0), stop=(c == 2))
                sq2 = gp.tile([P, 2 * F], F32, tag="sq2", bufs=3)
                nc.scalar.square(sq2[:], ri_ps[:])
                nc.gpsimd.tensor_add(mag[:, j, :], sq2[:, 0:F], sq2[:, F:2 * F])
            nc.scalar.sqrt(mag[:], mag[:])

            # ---- stage B: minmax, xn, l2 (batched per group) ----
            nc.vector.tensor_reduce(mx_s[:, sl], mag[:], axis=AX.X, op=OP.max)
            nc.vector.tensor_reduce(mn_s[:, sl], mag[:], axis=AX.X, op=OP.min)
            nc.vector.scalar_tensor_tensor(rd_s[:, sl], mx_s[:, sl], 1.0, mn_s[:, sl],
                                           op0=OP.add, op1=OP.subtract)
            nc.vector.reciprocal(rd_s[:, sl], rd_s[:, sl])
            xn_g = gp.tile([P, G, F], F32, tag="xn")
            for j in range(G):
                t = t0 + j
                nc.gpsimd.tensor_scalar(xn_g[:, j, :], mag[:, j, :],
                                        scalar1=mn_s[:, t:t + 1], scalar2=rd_s[:, t:t + 1],
                                        op0=OP.subtract, op1=OP.mult)
                scr = gp.tile([P, F], F32, tag="scrF")
                # (tensor_tensor_reduce mis-executes on HW; ACT square+accum)
                nc.scalar.activation(scr[:], xn_g[:, j, :], AF.Square,
                                     accum_out=ssx_s[:, t:t + 1])
            nc.scalar.sqrt(ssx_s[:, sl], ssx_s[:, sl])
            nc.vector.reciprocal(rnx_s[:, sl], ssx_s[:, sl])

            # ---- stage C: q = xn @ Wd0, cat, h = relu(cat @ We0) ----
            hr_g = gp.tile([P, G, H], F32, tag="hr")
            for j in range(G):
                t = t0 + j
                sc = ps_b.tile([P, 384], F32, tag="sc", bufs=3)
                nc.tensor.transpose(sc[:, 0:P], xn_g[:, j, 0:P], ident[:])
                nc.tensor.transpose(sc[0:F - P, P:2 * P], xn_g[:, j, P:F], ident[:])
                xnT_a = gp.tile([P, P], F32, tag="xnT_a")
                xnT_b = gp.tile([F - P, P], F32, tag="xnT_b")
                nc.vector.tensor_copy(xnT_a[:], sc[:, 0:P])
                nc.vector.tensor_copy(xnT_b[:], sc[0:F - P, P:2 * P])
                q_ps = sc[:, 2 * P:2 * P + H]
                nc.tensor.matmul(q_ps, lhsT=xnT_a[:], rhs=wd_a[:], start=True, stop=False)
                nc.tensor.matmul(q_ps, lhsT=xnT_b[:], rhs=wd_b[:], start=False, stop=True)
                cat_t = gp.tile([P, H + EMB], F32, tag="cat")
                nc.vector.tensor_scalar_mul(cat_t[:, 0:H], q_ps, rnx_s[:, t:t + 1])
                nc.gpsimd.tensor_scalar_mul(cat_t[:, H:H + EMB], ten_all[:, t, :],
                                            rnt_s[:, t:t + 1])
                pc = sc[0:H + EMB, P:2 * P]
                nc.tensor.transpose(pc, cat_t[:], ident[:])
                catT = gp.tile([H + EMB, P], F32, tag="catT")
                nc.vector.tensor_copy(catT[:], pc)
                h_ps = sc[:, 2 * P + H:2 * P + 2 * H]
                nc.tensor.matmul(h_ps, lhsT=catT[:], rhs=we_sb[:], start=True, stop=True)
                nc.vector.tensor_scalar(hr_g[:, j, :], h_ps, scalar1=0.0,
                                        scalar2=0.0, op0=OP.max, op1=OP.add,
                                        accum_out=sums_s[:, t:t + 1])

            # ---- stage D: LN + transpose into hT / hTw ----
            nc.vector.tensor_scalar_mul(mean_s[:, sl], sums_s[:, sl], -1.0 / H)
            for j in range(G):
                t = t0 + j
                scr = gp.tile([P, H], F32, tag="scrH")
                nc.scalar.activation(scr[:], hr_g[:, j, :], AF.Square,
                                     bias=mean_s[:, t:t + 1],
                                     accum_out=ssh_s[:, t:t + 1])
            nc.vector.tensor_scalar(ssh_s[:, sl], ssh_s[:, sl], scalar1=1.0 / H,
                                    scalar2=1e-8, op0=OP.mult, op1=OP.add)
            nc.scalar.sqrt(ssh_s[:, sl], ssh_s[:, sl])
            nc.vector.reciprocal(rstd_s[:, sl], ssh_s[:, sl])
            for j in range(G):
                t = t0 + j
                h_t = gp.tile([P, H], F32, tag="h_t")
                nc.vector.tensor_scalar(h_t[:], hr_g[:, j, :],
                                        scalar1=mean_s[:, t:t + 1],
                                        scalar2=rstd_s[:, t:t + 1],
                                        op0=OP.add, op1=OP.mult)
                hT_ps = ps_b.tile([H, P], F32, tag="sc", bufs=3)
                nc.tensor.transpose(hT_ps[:], h_t[:], ident[:])
                nc.vector.tensor_copy(hT_sb[:, t * P:(t + 1) * P], hT_ps[:])
                nc.gpsimd.tensor_scalar_mul(hTw_sb[:, t * P:(t + 1) * P],
                                            hT_sb[:, t * P:(t + 1) * P],
                                            w_sb[:, 0:1])

        p1ps.close()

        # ---- phase 2: adjacency + top-k + normalize ----
        # Decoupled pipeline: PSUM's only reader is a plain ACT copy (no
        # data-dependent latency), so the two PSUM slots recycle faster than
        # the PE's 4 fp32 matmuls per tile and the PE never stalls (pstate
        # stays ramped). Everything else runs from the SBUF copy: per-quarter
        # max8 candidates -> merge (max8+match_replace+max8) -> exact den =
        # sum(relu(top10)) + 1e-5 and thr = max(v10,tiny)/den; ACT re-copies
        # scaled by r; select on the scaled copy is split DVE stt (cols
        # 0:CD) / Pool mask+mult (cols CD:N).
        CD = 960
        with tc.tile_pool(name="p2_sb", bufs=5) as p2_sb, \
             tc.tile_pool(name="p2_sm", bufs=8) as p2_sm, \
             tc.tile_pool(name="p2_ps", bufs=2, space="PSUM") as p2_ps:
            for m in range(NT):
                adj_ps = p2_ps.tile([P, N], F32, tag="adj_ps")
                for n in range(4):
                    c0 = n * 512
                    nc.tensor.matmul(adj_ps[:, c0:c0 + 512],
                                     lhsT=hTw_sb[:, m * P:(m + 1) * P],
                                     rhs=hT_sb[:, c0:c0 + 512],
                                     start=True, stop=True)
                adjs = p2_sb.tile([P, N], F32, tag="adjs")
                nc.scalar.activation(adjs[:], adj_ps[:], AF.Copy)
                cand = p2_sm.tile([P, 32], F32, tag="cand")
                for q in range(4):
                    nc.vector.max(cand[:, q * 8:(q + 1) * 8],
                                  adjs[:, q * 512:(q + 1) * 512])
                mx16 = p2_sm.tile([P, 16], F32, tag="mx16")
                nc.vector.max(mx16[:, 0:8], cand[:])
                zap = p2_sm.tile([P, 32], F32, tag="zap")
                nc.vector.match_replace(zap[:], in_to_replace=mx16[:, 0:8],
                                        in_values=cand[:], imm_value=-3e38)
                nc.vector.max(mx16[:, 8:16], zap[:])
                top10r = p2_sm.tile([P, TOPK], F32, tag="top10r")
                nc.vector.tensor_scalar_max(top10r[:], mx16[:, 0:TOPK], 0.0)
                den = p2_sm.tile([P, 1], F32, tag="den")
                nc.vector.tensor_reduce(den[:], top10r[:], axis=AX.X, op=OP.add)
                nc.vector.tensor_scalar_add(den[:], den[:], 1e-5)
                r = p2_sm.tile([P, 1], F32, tag="r")
                nc.vector.reciprocal(r[:], den[:])
                thr = p2_sm.tile([P, 1], F32, tag="thr")
                nc.vector.scalar_tensor_tensor(thr[:], mx16[:, TOPK - 1:TOPK],
                                               1e-30, r[:], op0=OP.max,
                                               op1=OP.mult)
                adjr = p2_sb.tile([P, N], F32, tag="adjr")
                nc.scalar.activation(adjr[:], adjs[:], AF.Relu,
                                     scale=r[:, 0:1])
                outt = p2_sb.tile([P, N], F32, tag="outt")
                nc.vector.scalar_tensor_tensor(outt[:, 0:CD], adjr[:, 0:CD],
                                               thr[:, 0:1], adjr[:, 0:CD],
                                               op0=OP.is_ge, op1=OP.mult)
                msk = p2_sb.tile([P, N - CD], F32, tag="msk")
                nc.gpsimd.tensor_scalar(msk[:], adjr[:, CD:N],
                                        scalar1=thr[:, 0:1], scalar2=None,
                                        op0=OP.is_ge)
                nc.gpsimd.tensor_tensor(outt[:, CD:N], msk[:], adjr[:, CD:N],
                                        op=OP.mult)
                nc.sync.dma_start(out_d[m * P:(m + 1) * P, 0:CD],
                                  outt[:, 0:CD])
                nc.sync.dma_start(out_d[m * P:(m + 1) * P, CD:N],
                                  outt[:, CD:N])

    nc.compile()
    return nc


def _dft_mats():
    tt = np.arange(T)[:, None].astype(np.float64)
    kk = np.arange(F)[None, :].astype(np.float64)
    ang = 2.0 * np.pi * tt * kk / T
    s = 1.0 / np.sqrt(T)
    return (np.cos(ang) * s).astype(np.float32), (np.sin(ang) * s).astype(np.float32)


def kernel(x, t_emb, Wd0, We0, W):
    if "nc" not in _CACHE:
        _CACHE["nc"] = _build()
    nc = _CACHE["nc"]
    cc, cs = _dft_mats()
    base = {
        "ccos": cc, "csin": cs,
        "wd0": np.ascontiguousarray(Wd0, np.float32),
        "we0": np.ascontiguousarray(We0, np.float32),
        "w": np.ascontiguousarray(W, np.float32),
    }
    in_maps = [
        {**base,
         "x": np.ascontiguousarray(x[i], np.float32),
         "t_emb": np.ascontiguousarray(t_emb[i], np.float32)}
        for i in range(NCORES)
    ]
    res = run_bass_kernel_spmd(nc, in_maps, list(range(NCORES)))
    return np.stack([res.results[i]["out"] for i in range(NCORES)], axis=0)



# revision 41
# speedup vs baseline: 1.0409x; 1.0409x over previous
"""Trainium2 Bass kernel for nn_DFDgraph (gnn_message_passing).

Pipeline per batch element (one NeuronCore each, 8 total):
  x (2048, 288) --rfft-mag--> (2048, 145) --minmax+l2--> xn
  h = LN(relu(cat[xn @ Wd0, te_norm] @ We0))            (2048, 64)
  adj = relu((h * w) @ h^T)                             (2048, 2048)
  out = top10_row_mask(adj) / (rowsum_top10 + 1e-5)

Everything stays fp32: P(gap(v10,v11) < 1e-3) ~ 6.6% per row, so the
~1e-3 noise of f32r/bf16 matmuls flips enough top-k selections to blow
the error budget.

Phase 1 (row tiles of 128, groups of 4): PE transposes + DFT matmuls
against host-precomputed cos|sin matrices; one batched ACT square over
[128, 290]; group-batched minmax/l2/LN stats; elementwise work spread
across DVE/Pool/ACT; t_emb normalization fully hoisted out of the loop.
Stage C/D PSUM scratch is packed into one rotating 1-bank tile (pa/hT
and pb/pc share regions, ordered by within-tile deps).

Phase 2 per tile: 4 fp32 matmuls -> PSUM; the ONLY PSUM reader is a
plain ACT copy, so the two PSUM slots recycle fast and the PE never
stalls (pstate stays ramped). From the SBUF copy: per-quarter max8
gives 32 candidates (exact top-10 w.p. ~1-3e-4 miss/row), merge via
max8+match_replace+max8 -> exact top-16; den = sum(relu(top10)) + 1e-5
and thr = max(v10, tiny)/den are known BEFORE the select, so ACT
re-copies scaled by 1/den and the select (adjr >= thr)*adjr needs no
further scale pass. Select is split: DVE scalar_tensor_tensor on cols
0:CD, Pool mask+mult (stt is not supported on Pool; Pool cannot read
PSUM) on cols CD:N. relu is never materialized: negatives cannot pass
thr > 0, and rows with <10 positives are handled by den/thr clamping.

Known HW pitfalls baked in: tensor_tensor_reduce mis-executes on HW
(use ACT square+accum); fp32r matmul needs f32r-rounded producers and
is numerically unacceptable here anyway.
"""

import numpy as np
from contextlib import ExitStack

import concourse.bass as bass
import concourse.mybir as mybir
from concourse import bacc
from concourse import tile
from concourse import masks
from concourse.bass_utils import run_bass_kernel_spmd

F32 = mybir.dt.float32
AX = mybir.AxisListType
OP = mybir.AluOpType
AF = mybir.ActivationFunctionType

B, N, T, H, EMB, TOPK = 8, 2048, 288, 64, 24, 10
F = T // 2 + 1          # 145
P = 128                 # rows per tile
NT = N // P             # 16 tiles
G = 2                   # tiles per pipeline group
KC = 96                 # DFT contraction chunk (3 x 96 = 288)
NCORES = 8

_CACHE = {}


def _build():
    nc = bacc.Bacc("TRN2", target_bir_lowering=False, debug=False,
                   num_devices=NCORES)
    x_d = nc.declare_dram_parameter("x", [N, T], F32, isOutput=False)
    te_d = nc.declare_dram_parameter("t_emb", [N, EMB], F32, isOutput=False)
    cc_d = nc.declare_dram_parameter("ccos", [T, F], F32, isOutput=False)
    cs_d = nc.declare_dram_parameter("csin", [T, F], F32, isOutput=False)
    wd_d = nc.declare_dram_parameter("wd0", [F, H], F32, isOutput=False)
    we_d = nc.declare_dram_parameter("we0", [H + EMB, H], F32, isOutput=False)
    w_d = nc.declare_dram_parameter("w", [H, 1], F32, isOutput=False)
    out_d = nc.declare_dram_parameter("out", [N, N], F32, isOutput=True)

    with tile.TileContext(nc) as tc, ExitStack() as ctx:
        const = ctx.enter_context(tc.tile_pool(name="const", bufs=1))
        ident = const.tile([P, P], F32)
        masks.make_identity(nc, ident[:])
        ccs_sb = const.tile([KC, 3, 2 * F], F32)
        for c in range(3):
            nc.sync.dma_start(ccs_sb[:, c, 0:F], cc_d[c * KC:(c + 1) * KC, :])
            nc.sync.dma_start(ccs_sb[:, c, F:2 * F], cs_d[c * KC:(c + 1) * KC, :])
        wd_a = const.tile([P, H], F32)
        wd_b = const.tile([F - P, H], F32)
        we_sb = const.tile([H + EMB, H], F32)
        w_sb = const.tile([H, 1], F32)

        # persistent phase-1 results
        p1 = ctx.enter_context(tc.tile_pool(name="p1", bufs=1))
        hT_sb = p1.tile([H, N], F32)
        hTw_sb = p1.tile([H, N], F32)
        # [P, NT] stats, persistent
        st = ctx.enter_context(tc.tile_pool(name="stats", bufs=1))
        mx_s = st.tile([P, NT], F32)
        mn_s = st.tile([P, NT], F32)
        rd_s = st.tile([P, NT], F32)
        ssx_s = st.tile([P, NT], F32)
        rnx_s = st.tile([P, NT], F32)
        mxt_s = st.tile([P, NT], F32)
        mnt_s = st.tile([P, NT], F32)
        rdt_s = st.tile([P, NT], F32)
        sst_s = st.tile([P, NT], F32)
        rnt_s = st.tile([P, NT], F32)
        sums_s = st.tile([P, NT], F32)
        mean_s = st.tile([P, NT], F32)
        ssh_s = st.tile([P, NT], F32)
        rstd_s = st.tile([P, NT], F32)
        msq_s = st.tile([P, NT], F32)

        # t_emb pipeline hoisted out of the tile loop: one batched minmax +
        # l2 over [P, NT, EMB]; per-tile scalars applied on Pool.
        te_all = p1.tile([P, NT, EMB], F32)
        ten_all = p1.tile([P, NT, EMB], F32)
        for t in range(NT):
            nc.sync.dma_start(te_all[:, t, :], te_d[t * P:(t + 1) * P, :])
        nc.vector.tensor_reduce(mxt_s[:], te_all[:], axis=AX.X, op=OP.max)
        nc.vector.tensor_reduce(mnt_s[:], te_all[:], axis=AX.X, op=OP.min)
        nc.vector.scalar_tensor_tensor(rdt_s[:], mxt_s[:], 1.0, mnt_s[:],
                                       op0=OP.add, op1=OP.subtract)
        nc.vector.reciprocal(rdt_s[:], rdt_s[:])
        for t in range(NT):
            nc.gpsimd.tensor_scalar(ten_all[:, t, :], te_all[:, t, :],
                                    scalar1=mnt_s[:, t:t + 1],
                                    scalar2=rdt_s[:, t:t + 1],
                                    op0=OP.subtract, op1=OP.mult)
        sqt = p1.tile([P, NT, EMB], F32)
        nc.vector.tensor_tensor(sqt[:], ten_all[:], ten_all[:], op=OP.mult)
        nc.vector.tensor_reduce(sst_s[:], sqt[:], axis=AX.X, op=OP.add)
        nc.scalar.sqrt(sst_s[:], sst_s[:])
        nc.vector.reciprocal(rnt_s[:], sst_s[:])

        # group-cycled working buffers (bufs=2 -> group g+1 overlaps group g)
        p1ps = ExitStack()
        gp = p1ps.enter_context(tc.tile_pool(name="gp", bufs=4))
        ps_a = p1ps.enter_context(tc.tile_pool(name="ps_a", bufs=2, space="PSUM"))
        ps_b = p1ps.enter_context(tc.tile_pool(name="ps_b", bufs=2, space="PSUM"))

        for g in range(NT // G):
            t0 = g * G
            sl = slice(t0, t0 + G)
            # ---- stage A: load x, transpose, DFT, |.|^2, mag ----
            mag = gp.tile([P, G, F], F32, tag="mag")
            for j in range(G):
                t = t0 + j
                x_t = gp.tile([P, T], F32, tag="x", bufs=3)
                nc.sync.dma_start(x_t[:], x_d[t * P:(t + 1) * P, :])
                xTp = ps_a.tile([KC, 3, P], F32, tag="xT_ps")
                for c in range(3):
                    nc.tensor.transpose(xTp[:, c, :], x_t[:, c * KC:(c + 1) * KC],
                                        ident[:])
                xT = gp.tile([KC, 3, P], F32, tag="xT", bufs=3)
                nc.scalar.activation(xT[:], xTp[:], AF.Copy)
                ri_ps = ps_a.tile([P, 2 * F], F32, tag="ri_ps", bufs=3)
                for c in range(3):
                    nc.tensor.matmul(ri_ps[:], lhsT=xT[:, c, :], rhs=ccs_sb[:, c, :],
                                     start=(c == 0), stop=(c == 2))
                sq2 = gp.tile([P, 2 * F], F32, tag="sq2", bufs=3)
                nc.scalar.square(sq2[:], ri_ps[:])
                nc.gpsimd.tensor_add(mag[:, j, :], sq2[:, 0:F], sq2[:, F:2 * F])
            nc.scalar.sqrt(mag[:], mag[:])

            # ---- stage B: minmax, xn, l2 (batched per group) ----
            nc.vector.tensor_reduce(mx_s[:, sl], mag[:], axis=AX.X, op=OP.max)
            nc.vector.tensor_reduce(mn_s[:, sl], mag[:], axis=AX.X, op=OP.min)
            nc.vector.scalar_tensor_tensor(rd_s[:, sl], mx_s[:, sl], 1.0, mn_s[:, sl],
                                           op0=OP.add, op1=OP.subtract)
            nc.vector.reciprocal(rd_s[:, sl], rd_s[:, sl])
            xn_g = gp.tile([P, G, F], F32, tag="xn")
            for j in range(G):
                t = t0 + j
                nc.gpsimd.tensor_scalar(xn_g[:, j, :], mag[:, j, :],
                                        scalar1=mn_s[:, t:t + 1], scalar2=rd_s[:, t:t + 1],
                                        op0=OP.subtract, op1=OP.mult)
                scr = gp.tile([P, F], F32, tag="scrF")
                # (tensor_tensor_reduce mis-executes on HW; ACT square+accum)
                nc.scalar.activation(scr[:], xn_g[:, j, :], AF.Square,
                                     accum_out=ssx_s[:, t:t + 1])
            nc.scalar.sqrt(ssx_s[:, sl], ssx_s[:, sl])
            nc.vector.reciprocal(rnx_s[:, sl], ssx_s[:, sl])

            # ---- stage C: q = xn @ Wd0, cat, h = relu(cat @ We0) ----
            hr_g = gp.tile([P, G, H], F32, tag="hr")
            for j in range(G):
                t = t0 + j
                sc = ps_b.tile([P, 384], F32, tag="sc", bufs=3)
                nc.tensor.transpose(sc[:, 0:P], xn_g[:, j, 0:P], ident[:])
                nc.tensor.transpose(sc[0:F - P, P:2 * P], xn_g[:, j, P:F], ident[:])
                xnT_a = gp.tile([P, P], F32, tag="xnT_a")
                xnT_b = gp.tile([F - P, P], F32, tag="xnT_b")
                nc.vector.tensor_copy(xnT_a[:], sc[:, 0:P])
                nc.vector.tensor_copy(xnT_b[:], sc[0:F - P, P:2 * P])
                q_ps = sc[:, 2 * P:2 * P + H]
                nc.tensor.matmul(q_ps, lhsT=xnT_a[:], rhs=wd_a[:], start=True, stop=False)
                nc.tensor.matmul(q_ps, lhsT=xnT_b[:], rhs=wd_b[:], start=False, stop=True)
                cat_t = gp.tile([P, H + EMB], F32, tag="cat")
                nc.vector.tensor_scalar_mul(cat_t[:, 0:H], q_ps, rnx_s[:, t:t + 1])
                nc.gpsimd.tensor_scalar_mul(cat_t[:, H:H + EMB], ten_all[:, t, :],
                                            rnt_s[:, t:t + 1])
                pc = sc[0:H + EMB, P:2 * P]
                nc.tensor.transpose(pc, cat_t[:], ident[:])
                catT = gp.tile([H + EMB, P], F32, tag="catT")
                nc.vector.tensor_copy(catT[:], pc)
                h_ps = sc[:, 2 * P + H:2 * P + 2 * H]
                nc.tensor.matmul(h_ps, lhsT=catT[:], rhs=we_sb[:], start=True, stop=True)
                nc.vector.tensor_scalar(hr_g[:, j, :], h_ps, scalar1=0.0,
                                        scalar2=0.0, op0=OP.max, op1=OP.add,
                                        accum_out=sums_s[:, t:t + 1])
                scr2 = gp.tile([P, H], F32, tag="scrH")
                nc.vector.scalar_tensor_tensor(scr2[:], hr_g[:, j, :], 1.0,
                                               hr_g[:, j, :], op0=OP.mult,
                                               op1=OP.mult,
                                               accum_out=ssh_s[:, t:t + 1])

            # ---- stage D: LN + transpose into hT / hTw ----
            nc.vector.tensor_scalar_mul(mean_s[:, sl], sums_s[:, sl], -1.0 / H)
            # var = sum(hr^2)/H - mean^2 (moments; ACT Square pass deleted)
            nc.vector.tensor_mul(msq_s[:, sl], mean_s[:, sl], mean_s[:, sl])
            nc.vector.tensor_scalar(ssh_s[:, sl], ssh_s[:, sl], scalar1=1.0 / H,
                                    scalar2=1e-8, op0=OP.mult, op1=OP.add)
            nc.vector.tensor_tensor(ssh_s[:, sl], ssh_s[:, sl], msq_s[:, sl],
                                    op=OP.subtract)
            nc.scalar.sqrt(ssh_s[:, sl], ssh_s[:, sl])
            nc.vector.reciprocal(rstd_s[:, sl], ssh_s[:, sl])
            for j in range(G):
                t = t0 + j
                h_t = gp.tile([P, H], F32, tag="h_t")
                nc.vector.tensor_scalar(h_t[:], hr_g[:, j, :],
                                        scalar1=mean_s[:, t:t + 1],
                                        scalar2=rstd_s[:, t:t + 1],
                                        op0=OP.add, op1=OP.mult)
                hT_ps = ps_b.tile([H, P], F32, tag="sc", bufs=3)
                nc.tensor.transpose(hT_ps[:], h_t[:], ident[:])
                nc.vector.tensor_copy(hT_sb[:, t * P:(t + 1) * P], hT_ps[:])
                nc.gpsimd.tensor_scalar_mul(hTw_sb[:, t * P:(t + 1) * P],
                                            hT_sb[:, t * P:(t + 1) * P],
                                            w_sb[:, 0:1])

        p1ps.close()

        # ---- phase 2: adjacency + top-k + normalize ----
        # Decoupled pipeline: PSUM's only reader is a plain ACT copy (no
        # data-dependent latency), so the two PSUM slots recycle faster than
        # the PE's 4 fp32 matmuls per tile and the PE never stalls (pstate
        # stays ramped). Everything else runs from the SBUF copy: per-quarter
        # max8 candidates -> merge (max8+match_replace+max8) -> exact den =
        # sum(relu(top10)) + 1e-5 and thr = max(v10,tiny)/den; ACT re-copies
        # scaled by r; select on the scaled copy is split DVE stt (cols
        # 0:CD) / Pool mask+mult (cols CD:N).
        CD = 960
        with tc.tile_pool(name="p2_sb", bufs=5) as p2_sb, \
             tc.tile_pool(name="p2_sm", bufs=8) as p2_sm, \
             tc.tile_pool(name="p2_ps", bufs=2, space="PSUM") as p2_ps:
            for m in range(NT):
                adj_ps = p2_ps.tile([P, N], F32, tag="adj_ps")
                for n in range(4):
                    c0 = n * 512
                    nc.tensor.matmul(adj_ps[:, c0:c0 + 512],
                                     lhsT=hTw_sb[:, m * P:(m + 1) * P],
                                     rhs=hT_sb[:, c0:c0 + 512],
                                     start=True, stop=True)
                adjs = p2_sb.tile([P, N], F32, tag="adjs")
                nc.scalar.activation(adjs[:], adj_ps[:], AF.Copy)
                cand = p2_sm.tile([P, 32], F32, tag="cand")
                for q in range(4):
                    nc.vector.max(cand[:, q * 8:(q + 1) * 8],
                                  adjs[:, q * 512:(q + 1) * 512])
                mx16 = p2_sm.tile([P, 16], F32, tag="mx16")
                nc.vector.max(mx16[:, 0:8], cand[:])
                zap = p2_sm.tile([P, 32], F32, tag="zap")
                nc.vector.match_replace(zap[:], in_to_replace=mx16[:, 0:8],
                                        in_values=cand[:], imm_value=-3e38)
                nc.vector.max(mx16[:, 8:16], zap[:])
                top10r = p2_sm.tile([P, TOPK], F32, tag="top10r")
                nc.vector.tensor_scalar_max(top10r[:], mx16[:, 0:TOPK], 0.0)
                den = p2_sm.tile([P, 1], F32, tag="den")
                nc.vector.tensor_reduce(den[:], top10r[:], axis=AX.X, op=OP.add)
                nc.vector.tensor_scalar_add(den[:], den[:], 1e-5)
                r = p2_sm.tile([P, 1], F32, tag="r")
                nc.vector.reciprocal(r[:], den[:])
                thr = p2_sm.tile([P, 1], F32, tag="thr")
                nc.vector.scalar_tensor_tensor(thr[:], mx16[:, TOPK - 1:TOPK],
                                               1e-30, r[:], op0=OP.max,
                                               op1=OP.mult)
                adjr = p2_sb.tile([P, N], F32, tag="adjr")
                nc.scalar.activation(adjr[:], adjs[:], AF.Relu,
                                     scale=r[:, 0:1])
                outt = p2_sb.tile([P, N], F32, tag="outt")
                nc.vector.scalar_tensor_tensor(outt[:, 0:CD], adjr[:, 0:CD],
                                               thr[:, 0:1], adjr[:, 0:CD],
                                               op0=OP.is_ge, op1=OP.mult)
                msk = p2_sb.tile([P, N - CD], F32, tag="msk")
                nc.gpsimd.tensor_scalar(msk[:], adjr[:, CD:N],
                                        scalar1=thr[:, 0:1], scalar2=None,
                                        op0=OP.is_ge)
                nc.gpsimd.tensor_tensor(outt[:, CD:N], msk[:], adjr[:, CD:N],
                                        op=OP.mult)
                nc.sync.dma_start(out_d[m * P:(m + 1) * P, 0:CD],
                                  outt[:, 0:CD])
                nc.sync.dma_start(out_d[m * P:(m + 1) * P, CD:N],
                                  outt[:, CD:N])

    nc.compile()
    return nc


def _dft_mats():
    tt = np.arange(T)[:, None].astype(np.float64)
    kk = np.arange(F)[None, :].astype(np.float64)
    ang = 2.0 * np.pi * tt * kk / T
    s = 1.0 / np.sqrt(T)
    return (np.cos(ang) * s).astype(np.float32), (np.sin(ang) * s).astype(np.float32)


def kernel(x, t_emb, Wd0, We0, W):
    if "nc" not in _CACHE:
        _CACHE["nc"] = _build()
    nc = _CACHE["nc"]
    cc, cs = _dft_mats()
    base = {
        "ccos": cc, "csin": cs,
        "wd0": np.ascontiguousarray(Wd0, np.float32),
        "we0": np.ascontiguousarray(We0, np.float32),
        "w": np.ascontiguousarray(W, np.float32),
    }
    in_maps = [
        {**base,
         "x": np.ascontiguousarray(x[i], np.float32),
         "t_emb": np.ascontiguousarray(t_emb[i], np.float32)}
        for i in range(NCORES)
    ]
    res = run_bass_kernel_spmd(nc, in_maps, list(range(NCORES)))
    return np.stack([res.results[i]["out"] for i in range(NCORES)], axis=0)

